# revision 4
# baseline (speedup 1.0000x reference)
"""BiaffineSpan TRN2 kernel.

Full-input contract: kernel(**inputs) -> [B, L, L, C] float32.

Sharding: the C=256 bilinear channel dim is split across 8 NeuronCores
(32 channels each).  Each core computes, entirely on-device:

    Hs = MLP_s(hidden)            # [B, L, D]   (dup on every core)
    He = MLP_e(hidden)            # [B, L, D]
    T[c]  = Hs[b] @ W1[c]         # stage 1, per local channel
    S[c]  = T[c] @ He[b].T        # stage 2
    S[c] += Ls[b,i,c] + Le[b,j,c] + W2_b[c] + bias[c]

All matrices are kept transposed ([feature, token]) on-chip so both
matmul stages contract over the partition dim with no on-device
transposes.  Host side only reshapes/casts inputs and concatenates the
8 per-core [B, 32, L, L] outputs.

Numerics mode (matmul input dtype) via env BIAFFINE_MODE:
    bf16 (default) / f32r / f32
"""

import os
from contextlib import ExitStack

import numpy as np
import ml_dtypes

import concourse.bass as bass
import concourse.bacc as bacc
import concourse.mybir as mybir
import concourse.tile as tile
from concourse.bass_utils import run_bass_kernel_spmd

B, L, D, C = 2, 512, 768, 256
NCORES = 8
CLOC = C // NCORES          # 32 channels per core
T = B * L                   # 1024 tokens
P = 128
DT = D // P                 # 6 feature tiles
LT = L // P                 # 4 token tiles per batch el
TT = T // P                 # 8 token tiles total
NCH = T // 512              # 2 moving chunks of 512 tokens

F32 = mybir.dt.float32

MODE = os.environ.get("BIAFFINE_MODE", "bf16")


def _sb_dt():
    """SBUF storage dtype for matmul operands."""
    return mybir.dt.bfloat16 if MODE == "bf16" else mybir.dt.float32


def _np_dt():
    return ml_dtypes.bfloat16 if MODE == "bf16" else np.float32


def build_program():
    dt_s = _sb_dt()
    nc = bacc.Bacc("TRN2", target_bir_lowering=False, debug=False)

    # ---- DRAM parameters (per-core inputs) ----
    xT_h = nc.declare_dram_parameter("xT", [D, T], dt_s, isOutput=False)
    sw1T_h = nc.declare_dram_parameter("sw1T", [D, D], dt_s, isOutput=False)
    sw2T_h = nc.declare_dram_parameter("sw2T", [D, D], dt_s, isOutput=False)
    ew1T_h = nc.declare_dram_parameter("ew1T", [D, D], dt_s, isOutput=False)
    ew2T_h = nc.declare_dram_parameter("ew2T", [D, D], dt_s, isOutput=False)
    sb1_h = nc.declare_dram_parameter("sb1", [D], F32, isOutput=False)
    sb2_h = nc.declare_dram_parameter("sb2", [D], F32, isOutput=False)
    eb1_h = nc.declare_dram_parameter("eb1", [D], F32, isOutput=False)
    eb2_h = nc.declare_dram_parameter("eb2", [D], F32, isOutput=False)
    w1c_h = nc.declare_dram_parameter("w1c", [CLOC, D, D], dt_s, isOutput=False)
    wsT_h = nc.declare_dram_parameter("wsT", [D, CLOC], dt_s, isOutput=False)
    weT_h = nc.declare_dram_parameter("weT", [D, CLOC], dt_s, isOutput=False)
    w0_h = nc.declare_dram_parameter("w0", [CLOC, 1], F32, isOutput=False)
    out_h = nc.declare_dram_parameter("out", [B, CLOC, L, L], F32, isOutput=True)

    Relu = mybir.ActivationFunctionType.Relu
    Ident = mybir.ActivationFunctionType.Identity

    def mm(ps, lhsT, rhs, start, stop):
        if MODE == "f32r":
            lhsT = lhsT.bitcast(mybir.dt.float32r)
            rhs = rhs.bitcast(mybir.dt.float32r)
        nc.tensor.matmul(ps, lhsT, rhs, start=start, stop=stop)

    with tile.TileContext(nc) as tc, ExitStack() as ctx:
        # persistent pools
        p_h = ctx.enter_context(tc.tile_pool(name="hids", bufs=1))
        p_lin = ctx.enter_context(tc.tile_pool(name="lin", bufs=1))

        h1T = p_h.tile([P, DT, T], dt_s, tag="h1")
        hsT = p_h.tile([P, DT, T], dt_s, tag="hs")
        heT = p_h.tile([P, DT, T], dt_s, tag="he")

        # ---------------- Phase A: the two MLPs ----------------
        with (
            tc.tile_pool(name="ph_a", bufs=2) as p_a,
            tc.tile_pool(name="ps_a", bufs=4, space="PSUM") as ps_a,
            tc.tile_pool(name="bias", bufs=1) as p_bias,
        ):
            xT = p_a.tile([P, DT, T], dt_s, tag="x")
            nc.gpsimd.dma_start(
                out=xT[:], in_=xT_h[:].rearrange("(t p) n -> p t n", p=P)
            )
            b_sb = {}
            for nm, h in (("sb1", sb1_h), ("sb2", sb2_h),
                          ("eb1", eb1_h), ("eb2", eb2_h)):
                b_sb[nm] = p_bias.tile([P, DT], F32, tag=nm, name=nm)
                nc.gpsimd.dma_start(
                    out=b_sb[nm][:], in_=h[:].rearrange("(t p) -> p t", p=P)
                )

            def linear(inT, w_h, bias_t, outT, relu):
                wT = p_a.tile([P, DT, D], dt_s, tag="w_mlp")
                nc.gpsimd.dma_start(
                    out=wT[:], in_=w_h[:].rearrange("(t p) o -> p t o", p=P)
                )
                for ot in range(DT):
                    for chk in range(NCH):
                        ps = ps_a.tile([P, 512], F32, tag="ps_mlp")
                        for kt in range(DT):
                            mm(ps[:],
                               wT[:, kt, ot * P:(ot + 1) * P],
                               inT[:, kt, chk * 512:(chk + 1) * 512],
                               start=(kt == 0), stop=(kt == DT - 1))
                        nc.scalar.activation(
                            outT[:, ot, chk * 512:(chk + 1) * 512], ps[:],
                            Relu if relu else Ident,
                            bias=bias_t[:, ot:ot + 1])

            linear(xT, sw1T_h, b_sb["sb1"], h1T, relu=True)
            linear(h1T, sw2T_h, b_sb["sb2"], hsT, relu=False)
            linear(xT, ew1T_h, b_sb["eb1"], h1T, relu=True)
            linear(h1T, ew2T_h, b_sb["eb2"], heT, relu=False)

        # ---------------- Phase B: linear terms ----------------
        # LsP [tok_tile, 128, CLOC]  (token on partitions, channel free)
        # LeT [CLOC, T] (+ w0)      (channel on partitions, token free)
        lsP = p_lin.tile([P, TT, CLOC], F32, tag="lsP")
        leT = p_lin.tile([CLOC, T], dt_s, tag="leT")
        ones_t = p_lin.tile([1, P], dt_s, tag="ones")
        nc.vector.memset(ones_t[:], 1.0)

        with (
            tc.tile_pool(name="ps_b", bufs=2, space="PSUM") as ps_b,
            tc.tile_pool(name="wse", bufs=1) as p_wse,
        ):
            wsT = p_wse.tile([P, DT, CLOC], dt_s, tag="ws")
            weT = p_wse.tile([P, DT, CLOC], dt_s, tag="we")
            w0_sb = p_wse.tile([CLOC, 1], F32, tag="w0")
            nc.gpsimd.dma_start(
                out=wsT[:], in_=wsT_h[:].rearrange("(t p) c -> p t c", p=P))
            nc.gpsimd.dma_start(
                out=weT[:], in_=weT_h[:].rearrange("(t p) c -> p t c", p=P))
            nc.gpsimd.dma_start(out=w0_sb[:], in_=w0_h[:])

            for tt_ in range(TT):
                ps = ps_b.tile([P, CLOC], F32, tag="ps_ls")
                for kt in range(DT):
                    mm(ps[:],
                       hsT[:, kt, tt_ * P:(tt_ + 1) * P],
                       wsT[:, kt, :],
                       start=(kt == 0), stop=(kt == DT - 1))
                nc.vector.tensor_copy(lsP[:, tt_, :], ps[:])
            for chk in range(NCH):
                ps = ps_b.tile([CLOC, 512], F32, tag="ps_le")
                for kt in range(DT):
                    mm(ps[:],
                       weT[:, kt, :],
                       heT[:, kt, chk * 512:(chk + 1) * 512],
                       start=(kt == 0), stop=(kt == DT - 1))
                nc.vector.tensor_scalar_add(
                    leT[:, chk * 512:(chk + 1) * 512], ps[:], w0_sb[:])

        # ---------------- Phase C: main biaffine loop ----------------
        with (
            tc.tile_pool(name="w1c", bufs=2) as p_w1,
            tc.tile_pool(name="ttp", bufs=2) as p_tt,
            tc.tile_pool(name="lerow", bufs=2) as p_ler,
            tc.tile_pool(name="outp", bufs=6) as p_out,
            tc.tile_pool(name="ps_s1", bufs=4, space="PSUM") as ps_s1,
            tc.tile_pool(name="ps_s2", bufs=4, space="PSUM") as ps_s2,
        ):
            w1c_ap = w1c_h[:].rearrange("c (t p) e -> c p t e", p=P)
            out_ap = out_h[:]

            def stage1(w1t, b):
                tt_t = p_tt.tile([P, DT, 512], dt_s, tag="tt", name="tt_t")
                for et in range(DT):
                    ps = ps_s1.tile([P, 512], F32, tag="s1", name="ps1")
                    for dt_ in range(DT):
                        mm(ps[:],
                           w1t[:, dt_, et * P:(et + 1) * P],
                           hsT[:, dt_, b * 512:(b + 1) * 512],
                           start=(dt_ == 0), stop=(dt_ == DT - 1))
                    nc.vector.tensor_copy(tt_t[:, et, :], ps[:])
                return tt_t

            def stage2(tt_t, ler, c, b):
                for it in range(LT):
                    ps2 = ps_s2.tile([P, 512], F32, tag="s2", name="ps2")
                    for et in range(DT):
                        mm(ps2[:],
                           tt_t[:, et, it * P:(it + 1) * P],
                           heT[:, et, b * 512:(b + 1) * 512],
                           start=(et == 0), stop=False)
                    # rank-1: += 1 * (Le[b,:,c] + w0[c])
                    mm(ps2[:], ones_t[:], ler[:, b * 512:(b + 1) * 512],
                       start=False, stop=True)
                    o_t = p_out.tile([P, 512], F32, tag="o", name="o_t")
                    nc.scalar.activation(
                        o_t[:], ps2[:], Ident,
                        bias=lsP[:, b * LT + it, c:c + 1])
                    nc.sync.dma_start(
                        out=out_ap[b, c, it * P:(it + 1) * P, :],
                        in_=o_t[:])

            # Software-pipelined: stage1 of iteration k+1 is emitted before
            # stage2 of iteration k so PE never waits on TT evictions.
            pending = None
            w1t = ler = None
            for c in range(CLOC):
                w1t = p_w1.tile([P, DT, D], dt_s, tag="w1t", name="w1t")
                nc.gpsimd.dma_start(out=w1t[:], in_=w1c_ap[c])
                ler = p_ler.tile([1, T], dt_s, tag="ler", name="ler")
                nc.sync.dma_start(out=ler[:], in_=leT[c:c + 1, :])
                for b in range(B):
                    tt_t = stage1(w1t, b)
                    if pending is not None:
                        stage2(*pending)
                    pending = (tt_t, ler, c, b)
            stage2(*pending)
    nc.finalize()
    return nc


def _prep_inputs(inputs):
    """Host-side: transpose/cast/shard. Returns list of 8 in_maps."""
    npdt = _np_dt()
    f32 = np.float32

    def cast(x):
        return np.ascontiguousarray(x, dtype=npdt)

    h = np.asarray(inputs["hidden_states"], f32)
    xT = cast(h.reshape(T, D).T)
    sw1T = cast(np.asarray(inputs["sw1"], f32).T)
    sw2T = cast(np.asarray(inputs["sw2"], f32).T)
    ew1T = cast(np.asarray(inputs["ew1"], f32).T)
    ew2T = cast(np.asarray(inputs["ew2"], f32).T)
    sb1 = np.ascontiguousarray(inputs["sb1"], f32)
    sb2 = np.ascontiguousarray(inputs["sb2"], f32)
    eb1 = np.ascontiguousarray(inputs["eb1"], f32)
    eb2 = np.ascontiguousarray(inputs["eb2"], f32)
    W1 = np.asarray(inputs["W1"], f32)
    W2w = np.asarray(inputs["W2_w"], f32)
    wsT = np.asarray(W2w[:, :D].T)      # [D, C]
    weT = np.asarray(W2w[:, D:].T)      # [D, C]
    w0 = (np.asarray(inputs["W2_b"], f32)
          + np.asarray(inputs["bias"], f32)).reshape(C, 1)

    in_maps = []
    for m in range(NCORES):
        cs = slice(m * CLOC, (m + 1) * CLOC)
        in_maps.append({
            "xT": xT, "sw1T": sw1T, "sw2T": sw2T,
            "ew1T": ew1T, "ew2T": ew2T,
            "sb1": sb1, "sb2": sb2, "eb1": eb1, "eb2": eb2,
            "w1c": cast(W1[cs]),
            "wsT": cast(wsT[:, cs]),
            "weT": cast(weT[:, cs]),
            "w0": np.ascontiguousarray(w0[cs]),
        })
    return in_maps


def _gather(per_core_outs):
    full = np.concatenate(per_core_outs, axis=1)       # [B, C, L, L]
    return np.ascontiguousarray(full.transpose(0, 2, 3, 1))  # [B, L, L, C]


def kernel(**inputs):
    in_maps = _prep_inputs(inputs)
    nc = build_program()
    res = run_bass_kernel_spmd(nc, in_maps, list(range(NCORES)))
    return _gather([r["out"] for r in res.results])
